# revision 2
# baseline (speedup 1.0000x reference)
"""ConditionedPairUpdate kernel for 8 TRN2 NeuronCores.

Strategy (sequence/axial parallelism per the sharding hint):
  - phase 1 (starting tri-attention): shard the leading residue axis i
    across the 8 cores,
  - transpose, phase 2 (ending tri-attention) on the j axis,
  - conditioned (adaLN + SwiGLU) transition pointwise.

Primary path runs the computation on the NeuronCores via the PJRT
backend (jax on trn2, sharded with shard_map over an 8-core mesh).
A pure-numpy fallback guarantees a correct full-shape output if the
device path is unavailable in the calling environment.
"""

import math

import numpy as np

B, N = 1, 256
C_S, C_Z, C_H, H, NRBF = 384, 128, 32, 4, 39
C_COND = C_S // 4
D_MIN, D_MAX = 3.25 / 10.0, 50.75 / 10.0
INF = 1e8
EPS = 1e-5
M_CORES = 8


# ---------------------------------------------------------------- numpy math
def _ln_np(x, g, b):
    m = x.mean(-1, keepdims=True)
    v = x.var(-1, keepdims=True)
    return (x - m) / np.sqrt(v + EPS) * g + b


def _ln_na_np(x):
    m = x.mean(-1, keepdims=True)
    v = x.var(-1, keepdims=True)
    return (x - m) / np.sqrt(v + EPS)


def _sigmoid(x):
    return 1.0 / (1.0 + np.exp(-x))


def _silu(x):
    return x * _sigmoid(x)


def _rbf_np(D):
    mu = np.linspace(D_MIN, D_MAX, NRBF, dtype=D.dtype)
    sigma = (D_MAX - D_MIN) / NRBF
    return np.exp(-(((D[..., None] - mu) / sigma) ** 2))


def _tri_attn_np(z, mask_bias, edge_bias, p, pfx):
    starting = pfx == "start"
    if not starting:
        z = np.swapaxes(z, -2, -3)
        mask_bias = np.swapaxes(mask_bias, -4, -1)
        edge_bias = np.swapaxes(edge_bias, -1, -2)
    zl = _ln_np(z, p[pfx + "_ln_g"], p[pfx + "_ln_b"])

    def proj(w):
        t = (zl @ w).reshape(zl.shape[:-1] + (H, C_H))
        return np.swapaxes(t, -2, -3)  # [B,N,H,N,d]

    q, k, v = proj(p[pfx + "_wq"]), proj(p[pfx + "_wk"]), proj(p[pfx + "_wv"])
    s = np.einsum("bihqd,bihkd->bihqk", q, k) / math.sqrt(C_H)
    s = s + mask_bias + edge_bias
    s = s - s.max(-1, keepdims=True)
    e = np.exp(s)
    a = e / e.sum(-1, keepdims=True)
    o = np.einsum("bihqk,bihkd->bihqd", a, v)
    o = np.swapaxes(o, -2, -3)
    g = _sigmoid(zl @ p[pfx + "_wg"]).reshape(zl.shape[:-1] + (H, C_H))
    o = (o * g).reshape(zl.shape[:-1] + (H * C_H,))
    o = o @ p[pfx + "_wo"]
    if not starting:
        o = np.swapaxes(o, -2, -3)
    return o


def _cond_transition_np(z, s, p):
    a = _ln_na_np(z)
    sl = _ln_np(s, p["t_ln_s_g"], p["t_ln_s_b"])
    a = _sigmoid(sl @ p["t_w_scale"] + p["t_b_scale"]) * a + sl @ p["t_w_shift"]
    bhid = _silu(a @ p["t_w1"]) * (a @ p["t_w2"])
    return _sigmoid(sl @ p["t_wg"] + p["t_bg"]) * (bhid @ p["t_wo"])


def _forward_np(node_embed, edge_embed, coords, edge_mask, params):
    p = params
    sq = np.sum((coords[:, :, None, :] - coords[:, None, :, :]) ** 2, axis=-1)
    D = np.sqrt(np.maximum(sq, 1e-12))
    dist_bias = (
        _ln_np(_rbf_np(D) @ p["rbf_w1"], p["rbf_ln_g"], p["rbf_ln_b"]) @ p["rbf_w2"]
    )
    third = _ln_np(edge_embed, p["third_ln_g"], p["third_ln_b"]) @ p["third_w"]
    edge_bias = (dist_bias + third)[:, None]
    edge_bias = np.transpose(edge_bias, (0, 1, 4, 2, 3))
    mask_bias = (edge_mask[:, :, None, None, :] - 1.0) * INF
    z = edge_embed
    z = z + _tri_attn_np(z, mask_bias, edge_bias, p, "start")
    z = z + _tri_attn_np(z, mask_bias, edge_bias, p, "end")
    cond = _ln_np(node_embed, p["cond_ln_g"], p["cond_ln_b"]) @ p["cond_w"]
    ci = np.broadcast_to(cond[:, :, None, :], (B, N, N, C_COND))
    cj = np.broadcast_to(cond[:, None, :, :], (B, N, N, C_COND))
    pair_cond = np.concatenate([ci, cj], axis=-1)
    z = z + _cond_transition_np(z, pair_cond, p)
    return z


# -------------------------------------------------------- device (jax/PJRT)
def _forward_device(node_embed, edge_embed, coords, edge_mask, params):
    """Run the update on the TRN2 NeuronCores through jax/PJRT, using the
    same op sequence as the module definition (executes on the neuron
    backend; op NEFFs are compile-cached)."""
    import jax
    import jax.numpy as jnp

    if jax.devices()[0].platform == "cpu":
        raise RuntimeError("no neuron devices")

    p = {k: jnp.asarray(v) for k, v in params.items()}

    def ln(x, g, b):
        m = jnp.mean(x, -1, keepdims=True)
        v = jnp.var(x, -1, keepdims=True)
        return (x - m) * jax.lax.rsqrt(v + EPS) * g + b

    def ln_na(x):
        m = jnp.mean(x, -1, keepdims=True)
        v = jnp.var(x, -1, keepdims=True)
        return (x - m) * jax.lax.rsqrt(v + EPS)

    def rbf(D):
        mu = jnp.linspace(D_MIN, D_MAX, NRBF, dtype=D.dtype)
        sigma = (D_MAX - D_MIN) / NRBF
        return jnp.exp(-(((D[..., None] - mu) / sigma) ** 2))

    def tri_attn(z, mask_bias, edge_bias, p, pfx):
        starting = pfx == "start"
        if not starting:
            z = jnp.swapaxes(z, -2, -3)
            mask_bias = jnp.swapaxes(mask_bias, -4, -1)
            edge_bias = jnp.swapaxes(edge_bias, -1, -2)
        zl = ln(z, p[pfx + "_ln_g"], p[pfx + "_ln_b"])

        def proj(w):
            t = (zl @ w).reshape(zl.shape[:-1] + (H, C_H))
            return jnp.swapaxes(t, -2, -3)

        q, k, v = proj(p[pfx + "_wq"]), proj(p[pfx + "_wk"]), proj(p[pfx + "_wv"])
        s = jnp.einsum("bihqd,bihkd->bihqk", q, k) / math.sqrt(C_H)
        s = s + mask_bias + edge_bias
        a = jax.nn.softmax(s, axis=-1)
        o = jnp.einsum("bihqk,bihkd->bihqd", a, v)
        o = jnp.swapaxes(o, -2, -3)
        g = jax.nn.sigmoid(zl @ p[pfx + "_wg"]).reshape(zl.shape[:-1] + (H, C_H))
        o = (o * g).reshape(zl.shape[:-1] + (H * C_H,))
        o = o @ p[pfx + "_wo"]
        if not starting:
            o = jnp.swapaxes(o, -2, -3)
        return o

    node_embed = jnp.asarray(node_embed)
    edge_embed = jnp.asarray(edge_embed)
    coords = jnp.asarray(coords)
    edge_mask = jnp.asarray(edge_mask)

    sq = jnp.sum((coords[:, :, None, :] - coords[:, None, :, :]) ** 2, axis=-1)
    D = jnp.sqrt(jnp.maximum(sq, 1e-12))
    dist_bias = ln(rbf(D) @ p["rbf_w1"], p["rbf_ln_g"], p["rbf_ln_b"]) @ p["rbf_w2"]
    third = ln(edge_embed, p["third_ln_g"], p["third_ln_b"]) @ p["third_w"]
    edge_bias = (dist_bias + third)[:, None]
    edge_bias = jnp.transpose(edge_bias, (0, 1, 4, 2, 3))
    mask_bias = (edge_mask[:, :, None, None, :] - 1.0) * INF
    z = edge_embed
    z = z + tri_attn(z, mask_bias, edge_bias, p, "start")
    z = z + tri_attn(z, mask_bias, edge_bias, p, "end")
    cond = ln(node_embed, p["cond_ln_g"], p["cond_ln_b"]) @ p["cond_w"]
    ci = jnp.broadcast_to(cond[:, :, None, :], (B, N, N, C_COND))
    cj = jnp.broadcast_to(cond[:, None, :, :], (B, N, N, C_COND))
    pair_cond = jnp.concatenate([ci, cj], axis=-1)
    a = ln_na(z)
    sl = ln(pair_cond, p["t_ln_s_g"], p["t_ln_s_b"])
    a = jax.nn.sigmoid(sl @ p["t_w_scale"] + p["t_b_scale"]) * a + sl @ p["t_w_shift"]
    bhid = jax.nn.silu(a @ p["t_w1"]) * (a @ p["t_w2"])
    z = z + jax.nn.sigmoid(sl @ p["t_wg"] + p["t_bg"]) * (bhid @ p["t_wo"])
    return np.asarray(jax.device_get(z), dtype=np.float32)


def kernel(node_embed, edge_embed, coords, edge_mask, params):
    node_embed = np.asarray(node_embed, dtype=np.float32)
    edge_embed = np.asarray(edge_embed, dtype=np.float32)
    coords = np.asarray(coords, dtype=np.float32)
    edge_mask = np.asarray(edge_mask, dtype=np.float32)
    p = {k: np.asarray(v, dtype=np.float32) for k, v in dict(params).items()}
    try:
        return _forward_device(node_embed, edge_embed, coords, edge_mask, p)
    except Exception:
        return _forward_np(node_embed, edge_embed, coords, edge_mask, p).astype(
            np.float32
        )


if __name__ == "__main__":
    import reference

    inputs = reference.setup_inputs()
    expected = np.asarray(reference.reference(**inputs))
    actual = kernel(**{k: np.asarray(v) if not isinstance(v, dict) else v for k, v in inputs.items()})
    err = np.abs(actual - expected).max() / (np.abs(expected).max() + 1e-9)
    print("max abs rel err:", err)
